# revision 1
# baseline (speedup 1.0000x reference)
"""AngularAggLayer Trainium2 kernel — 8-core row-sharded.

Strategy: kernel() receives full inputs. Host (numpy) does the cheap O(N*D)
prep: normalized features, class centers, fake labels, the [C,C] angle table,
and per-core input slabs (A transposed so the device contracts along SBUF
partitions). Each of the 8 NeuronCores computes its 768-row slice of the
output: build the angle-modulated complex adjacency for its slice (table
select via small matmuls + Sin activations) and the complex message matmul
(nf.T @ adjT, accumulated over 48 k-tiles of 128), then normalizes to unit
modulus. Host reassembles the [6144, 128] complex64 output.
"""

import numpy as np

N, D, C = 6144, 128, 16
NCORES = 8
NS = N // NCORES          # 768 rows per core
KT = N // 128             # 48 contraction tiles
MC = 384                  # matmul free-dim chunk (2 chunks of 384 = NS)
EPS = np.float32(1e-5)

_CACHE = {}


def _legalize_waits(nc, mybir, max_waits=1):
    """Walrus in this container accepts only one sem wait per instruction;
    spill extras onto NoOps inserted just before, on the same engine."""
    ctr = 0
    for f in nc.m.functions:
        for bb in f.blocks:
            out, changed = [], False
            for inst in bb.instructions:
                si = inst.sync_info
                waits = list(si.on_wait) if si is not None and si.on_wait else []
                if len(waits) > max_waits:
                    while len(waits) > max_waits:
                        chunk, waits = waits[:max_waits], waits[max_waits:]
                        nop = mybir.InstNoOp(name=f"waitnop-{ctr}", ins=[], outs=[])
                        ctr += 1
                        nop.engine = inst.engine
                        nop.sync_info = mybir.SyncInfo(on_wait=chunk, on_update=[])
                        out.append(nop)
                    si.on_wait = waits
                    changed = True
                out.append(inst)
            if changed:
                bb.instructions = out


def _build(legalize=True):
    import concourse.bass as bass
    import concourse.mybir as mybir
    from concourse import tile

    F32 = mybir.dt.float32
    F32R = mybir.dt.float32r
    BF16 = mybir.dt.bfloat16
    AF = mybir.ActivationFunctionType
    ALU = mybir.AluOpType
    PI = float(np.pi)

    nc = bass.Bass()
    at_d = nc.declare_dram_parameter("at", [N, NS], BF16, isOutput=False)
    nfr_d = nc.declare_dram_parameter("nfr", [N, D], F32, isOutput=False)
    nfi_d = nc.declare_dram_parameter("nfi", [N, D], F32, isOutput=False)
    ekt_d = nc.declare_dram_parameter("ekt", [C, N], F32, isOutput=False)
    cmc_d = nc.declare_dram_parameter("cmc", [C, NS], F32, isOutput=False)
    cms_d = nc.declare_dram_parameter("cms", [C, NS], F32, isOutput=False)
    colr_d = nc.declare_dram_parameter("colr", [D, 1], F32, isOutput=False)
    coli_d = nc.declare_dram_parameter("coli", [D, 1], F32, isOutput=False)
    outr_d = nc.declare_dram_parameter("outr", [D, NS], F32, isOutput=True)
    outi_d = nc.declare_dram_parameter("outi", [D, NS], F32, isOutput=True)

    with tile.TileContext(nc) as tc:
        with (
            tc.tile_pool(name="const", bufs=1) as const,
            tc.tile_pool(name="atp", bufs=3) as atp,
            tc.tile_pool(name="angp", bufs=2) as angp,
            tc.tile_pool(name="adjp", bufs=2) as adjp,
            tc.tile_pool(name="outp", bufs=2) as outp,
            tc.tile_pool(name="psA", bufs=2, space="PSUM") as psA,
            tc.tile_pool(name="psM", bufs=1, space="PSUM") as psM,
        ):
            # ---- prologue: load + convert operand planes ----
            nfr_r = const.tile([128, KT, D], F32R)
            nfi_r = const.tile([128, KT, D], F32R)
            nfin_r = const.tile([128, KT, D], F32R)
            ektb = const.tile([C, N], F32R)
            cmcb = const.tile([C, NS], F32R)
            cmsb = const.tile([C, NS], F32R)
            colr_t = const.tile([D, 1], F32)
            coli_t = const.tile([D, 1], F32)
            nc.sync.dma_start(colr_t[:], colr_d[:])
            nc.sync.dma_start(coli_t[:], coli_d[:])
            with tc.tile_pool(name="stage", bufs=2) as stage:
                nfr_st = stage.tile([128, KT, D], F32, tag="st", name="nfr_st")
                nc.sync.dma_start(nfr_st[:], nfr_d.rearrange("(t p) d -> p t d", p=128))
                nc.scalar.copy(nfr_r[:], nfr_st[:])
                nfi_st = stage.tile([128, KT, D], F32, tag="st", name="nfi_st")
                nc.sync.dma_start(nfi_st[:], nfi_d.rearrange("(t p) d -> p t d", p=128))
                nc.scalar.copy(nfi_r[:], nfi_st[:])
                nc.scalar.mul(nfin_r[:], nfi_st[:], -1.0)
                ekt_st = stage.tile([C, N], F32, tag="st", name="ekt_st")
                nc.sync.dma_start(ekt_st[:], ekt_d[:])
                nc.scalar.copy(ektb[:], ekt_st[:])

                cmc_st = stage.tile([C, NS], F32, tag="stc", name="cmc_st")
                cms_st = stage.tile([C, NS], F32, tag="stc", name="cms_st")
                nc.sync.dma_start(cmc_st[:], cmc_d[:])
                nc.sync.dma_start(cms_st[:], cms_d[:])
                nc.vector.tensor_copy(cmcb[:], cmc_st[:])
                nc.vector.tensor_copy(cmsb[:], cms_st[:])

            # ---- persistent accumulators: message.T planes ----
            ps_r = [psM.tile([128, MC], F32, tag=f"psr{c}", name=f"psr{c}") for c in range(2)]
            ps_i = [psM.tile([128, MC], F32, tag=f"psi{c}", name=f"psi{c}") for c in range(2)]

            def emit_front(k):
                """DMA + table select + mask for k-tile k; returns adj tiles."""
                ks = slice(k * 128, (k + 1) * 128)
                at_t = atp.tile([128, NS], BF16, tag="at", name="at_t")
                nc.sync.dma_start(at_t[:], at_d[ks, :])
                adj = []
                for c in range(2):
                    cs = slice(c * MC, (c + 1) * MC)
                    sel_c = psA.tile([128, 1024], F32, tag=f"sel{c}",
                                     name=f"sel{c}", bufs=1)
                    nc.tensor.matmul(sel_c[:, 0:MC], ektb[:, ks], cmcb[:, cs],
                                     start=True, stop=True)
                    nc.tensor.matmul(sel_c[:, 512:512 + MC], ektb[:, ks],
                                     cmsb[:, cs], start=True, stop=True)
                    # adj_c[:,0,:] = mask*(cosW-1); adj_c[:,1,:] = mask*sinW
                    a_c = adjp.tile([128, 2, MC], F32R, tag=f"adj{c}",
                                    name=f"adj{c}")
                    selv = sel_c.rearrange("p (two x) -> p two x", two=2)[:, :, 0:MC]
                    atv = at_t[:, None, cs].to_broadcast((128, 2, MC))
                    nc.vector.scalar_tensor_tensor(
                        a_c[:], atv, 0.0, selv,
                        op0=ALU.is_gt, op1=ALU.mult)
                    adj.append(a_c)
                return adj

            def emit_big(k, adj):
                # message.T += nf[k].T @ adjT[k]  (the +1 of the real plane is
                # folded into a column-sum correction in the epilogue)
                first, last = (k == 0), (k == KT - 1)
                for c in range(2):
                    nc.tensor.matmul(ps_r[c][:], nfr_r[:, k, :], adj[c][:, 0, :],
                                     start=first, stop=False)
                    nc.tensor.matmul(ps_r[c][:], nfin_r[:, k, :], adj[c][:, 1, :],
                                     start=False, stop=last)
                    nc.tensor.matmul(ps_i[c][:], nfi_r[:, k, :], adj[c][:, 0, :],
                                     start=first, stop=False)
                    nc.tensor.matmul(ps_i[c][:], nfr_r[:, k, :], adj[c][:, 1, :],
                                     start=False, stop=last)

            # software pipeline: front(k) overlaps big(k-1) on the PE
            prev = emit_front(0)
            for k in range(1, KT):
                cur = emit_front(k)
                emit_big(k - 1, prev)
                prev = cur
            emit_big(KT - 1, prev)

            # ---- epilogue: normalize to unit modulus, store ----
            for c in range(2):
                cs = slice(c * MC, (c + 1) * MC)
                tr = outp.tile([128, MC], F32, tag="tr")
                ti = outp.tile([128, MC], F32, tag="ti")
                nc.vector.tensor_scalar_add(tr[:], ps_r[c][:], colr_t[:])
                nc.vector.tensor_scalar_add(ti[:], ps_i[c][:], coli_t[:])
                r2 = outp.tile([128, MC], F32, tag="r2")
                i2 = outp.tile([128, MC], F32, tag="i2")
                nc.scalar.square(r2[:], tr[:])
                nc.scalar.square(i2[:], ti[:])
                m2 = outp.tile([128, MC], F32, tag="m2")
                nc.vector.tensor_add(m2[:], r2[:], i2[:])
                mag = outp.tile([128, MC], F32, tag="mag")
                nc.scalar.sqrt(mag[:], m2[:])
                den = outp.tile([128, MC], F32, tag="den")
                nc.vector.tensor_scalar_add(den[:], mag[:], float(EPS))
                rec = outp.tile([128, MC], F32, tag="rec")
                nc.vector.reciprocal(rec[:], den[:])
                orr = outp.tile([128, MC], F32, tag="orr")
                oii = outp.tile([128, MC], F32, tag="oii")
                nc.vector.tensor_mul(orr[:], tr[:], rec[:])
                nc.vector.tensor_mul(oii[:], ti[:], rec[:])
                nc.sync.dma_start(outr_d[:, cs], orr[:])
                nc.sync.dma_start(outi_d[:, cs], oii[:])

    if legalize:
        _legalize_waits(nc, mybir)
    return nc


def _get_nc():
    if "nc" not in _CACHE:
        _CACHE["nc"] = _build()
    return _CACHE["nc"]


def kernel(x_real, x_imag, A, theta, params_real, params_imag, labels):
    import ml_dtypes
    from concourse.bass_utils import run_bass_kernel_spmd

    x_real = np.asarray(x_real, np.float32)
    x_imag = np.asarray(x_imag, np.float32)
    A = np.asarray(A, np.float32)
    theta = np.asarray(theta, np.float32)
    labels = np.asarray(labels)

    # --- host prep (mirrors reference order in float32) ---
    x = (x_real + 1j * x_imag).astype(np.complex64)
    nf = x / (np.abs(x) + EPS)                      # [N, D] complex64
    one_hot = np.zeros((N, C), np.float32)
    one_hot[np.arange(N), labels] = 1.0
    sum_by_label = np.einsum("nc,nd->cd", one_hot.astype(np.complex64), nf)
    counts = one_hot.sum(axis=0)[:, None]
    mean_tensor = sum_by_label / counts             # [C, D] complex64

    params = (np.asarray(params_real, np.float32)
              + 1j * np.asarray(params_imag, np.float32)).astype(np.complex64)
    p1, p2 = params[:D], params[D:]
    s_feat = nf @ p1                                # [N, 1]
    s_cent = mean_tensor @ p2                       # [C, 1]
    scores = np.abs(s_feat[:, None, :] + s_cent[None, :, :])[..., 0]
    fl = np.argmax(scores, axis=1)                  # [N] fake labels

    iu = np.triu_indices(C, k=1)
    il = np.tril_indices(C, k=-1)
    M = np.zeros((C, C), np.float32)
    M[iu[0], iu[1]] = theta
    M[il[1], il[0]] = -theta
    Mcos = np.cos(M) - np.float32(1.0)   # cos(W)-1 table (the +1 is folded
    Msin = np.sin(M)                     # into a colsum epilogue correction)

    ekt = np.zeros((C, N), np.float32)
    ekt[fl, np.arange(N)] = 1.0

    nfr = np.ascontiguousarray(nf.real)
    nfi = np.ascontiguousarray(nf.imag)
    colr = nfr.sum(axis=0, dtype=np.float64).astype(np.float32)[:, None]
    coli = nfi.sum(axis=0, dtype=np.float64).astype(np.float32)[:, None]

    in_maps = []
    for cid in range(NCORES):
        rows = slice(cid * NS, (cid + 1) * NS)
        cmc = np.ascontiguousarray(Mcos[fl[rows], :].T)       # [C, NS]
        cms = np.ascontiguousarray(Msin[fl[rows], :].T)
        at = np.asarray(np.ascontiguousarray(A[rows, :].T), ml_dtypes.bfloat16)
        in_maps.append(dict(at=at, nfr=nfr, nfi=nfi, ekt=ekt, cmc=cmc,
                            cms=cms, colr=colr, coli=coli))

    nc = _get_nc()
    _CACHE["last_maps"] = in_maps
    res = run_bass_kernel_spmd(nc, in_maps, list(range(NCORES))).results

    out = np.empty((N, D), np.complex64)
    for cid in range(NCORES):
        rows = slice(cid * NS, (cid + 1) * NS)
        out[rows] = (res[cid]["outr"].T + 1j * res[cid]["outi"].T)
    return out

